# revision 43
# baseline (speedup 1.0000x reference)
"""Trainium2 Bass kernel for nn_AggregationLayer (per-class masked reductions + Hough voting).

Strategy (8 NeuronCores, data-parallel over batch: 2 samples/core):
  The device computes, per (class c in 1..6, sample b), 13 masked sums
      S_c[x] = sum_p [cat_p == c] * x_p
  over the 307200 pixels of each sample, for channels x in
      {1, q0..q3, s0..s2, z, dxh2, m, u, v}
  where dxh2 = dx^2, m = dx*dy (direction-normalized via rsqrt(|xy|^2+delta))
  are the Hough direction-matrix terms (dy^2 = 1 - dx^2 is folded into the
  host finish) and
      u = dy^2*pu - m*pv,  v = dx^2*pv - m*pu
  fold the position-weighted Hough rhs into single channels:
  rx = sum mask*u, ry = sum mask*v exactly.

  The segmented (per-class) reduction runs on the TensorEngine with G=20
  pixel-chunks fused per matmul: stationary = 20 chunks' one-hot columns
  [128, 120] (chunk-major layout -> contiguous run), moving = the 13 channel
  planes' 20-column runs as a 2D channel-outer AP [128, 13, 20] (innermost
  contiguous), accumulated into a [120, 260] PSUM tile whose (g,g) diagonal
  blocks are the wanted per-class sums (off-diagonal cross products are
  ignored). 240 matmuls/core at ~95 ns each; LDWEIGHTS overlaps streaming.

  One-hot masks are built in chunk-major layout with a SINGLE is_equal
  tensor_tensor per slab: cat broadcast along an inner 6-wide dim vs a
  periodic [1..6] class pattern - all accesses contiguous.

  The host does only the tiny [6, B] finalization: 2x2 solve for the Hough
  center, quaternion -> rotation matrix, intrinsics backprojection, packing
  into the [6, 16, 26] output.
"""

import numpy as np
import ml_dtypes

B, H, W = 16, 480, 640
CLASSES = 7
C1 = CLASSES - 1
NCORES = 8
SPC = B // NCORES          # samples per core
NPART = 128
COLS = (H * W) // NPART    # 2400
SLAB = 400
NSLAB = COLS // SLAB       # 3
NCH = 13                   # moving channels
GRP = 20                   # pixel chunks fused per matmul
DELTA = 1e-12              # guard for 1/(n2 + DELTA)
EPS = 1e-6                 # matches reference

BF16 = ml_dtypes.bfloat16
FP8E4 = ml_dtypes.float8_e4m3fn

# moving-channel slot map
S_ONE, S_Q, S_S, S_Z = 0, 1, 5, 8
S_DXH2, S_M = 9, 10
S_U, S_V = 11, 12

_NC_CACHE = {}
_STATIC_CACHE = {}


def _build_static():
    if "st" in _STATIC_CACHE:
        return _STATIC_CACHE["st"]
    p = np.arange(H * W, dtype=np.int64)
    # pu/pv pre-scaled by 1/4 so |u| = |dy^2*pu - m*pv| fits fp8e4's range;
    # the host finish multiplies rx/ry back by 4
    pu = (p % W).astype(np.float64) * 0.25
    pv = (p // W).astype(np.float64) * 0.25
    ones = np.ones_like(pu)

    def plane16(a):
        return a.reshape(NPART, COLS).astype(BF16)

    st16 = np.stack([plane16(pu), plane16(pv)])  # [2,128,2400] bf16
    st8 = ones.reshape(NPART, COLS).astype(FP8E4)
    _STATIC_CACHE["st"] = (st16, st8)
    return st16, st8


def _build_nc(reps=1, skip=(), unroll=1):
    """Build + compile the SPMD Bass program. reps > 1 wraps the whole
    pipeline in a hardware For loop (used only for benchmarking; the loop
    body holds `unroll` reps and the loop runs reps//unroll times).
    skip: subset of {"mask","chan","mm","dma"} disabling stages (timing only)."""
    skip = frozenset(skip)
    key = (reps, SLAB, GRP, NCH, skip, unroll)
    if key in _NC_CACHE:
        return _NC_CACHE[key]
    import contextlib
    import concourse.bacc as bacc
    import concourse.mybir as mybir
    import concourse.tile as tile

    F32, MBF16 = mybir.dt.float32, mybir.dt.bfloat16
    FP8 = mybir.dt.float8e4
    AOT = mybir.AluOpType
    ACTF = mybir.ActivationFunctionType

    NG = SLAB // GRP           # matmul groups per slab
    SCOL = GRP * C1            # stationary columns (120)
    MCOL = GRP * NCH           # moving columns (260)

    nc = bacc.Bacc("TRN2", target_bir_lowering=False, debug=False)
    feat_d = nc.dram_tensor("feat", [SPC, 8, NPART, COLS], FP8, kind="ExternalInput")
    ohm_d = nc.dram_tensor("ohm", [SPC, NPART, C1 * H * W // NPART], FP8,
                           kind="ExternalInput")
    xy_d = nc.dram_tensor("xy", [SPC, 2, NPART, COLS], MBF16, kind="ExternalInput")
    st8_d = nc.dram_tensor("st8", [NPART, COLS], FP8, kind="ExternalInput")
    st16_d = nc.dram_tensor("st16", [2, NPART, COLS], MBF16, kind="ExternalInput")
    sums_d = nc.dram_tensor("sums", [SPC, SCOL, MCOL], F32, kind="ExternalOutput")

    with tile.TileContext(nc) as tc:
        with (
            tc.tile_pool(name="mov", bufs=1) as pmov,
            tc.tile_pool(name="stat", bufs=1) as pstat,
            tc.tile_pool(name="tmp", bufs=3) as ptmp,
            tc.tile_pool(name="psum", bufs=1, space="PSUM") as pps,
        ):
            # persistent buffers, one per slab phase (3-deep rotation);
            # the static ones channel is written once per physical buffer and
            # survives the per-sample rewrites of slots 1-12
            m_bufs = [pmov.tile([NPART, NCH * SLAB], FP8, name=f"Mbuf{k}",
                                tag=f"Mbuf{k}") for k in range(NSLAB)]
            oh_bufs = [pmov.tile([NPART, C1 * SLAB], FP8, name=f"OH{k}",
                                 tag=f"OH{k}") for k in range(NSLAB)]
            xy_bufs = [pmov.tile([NPART, 2 * SLAB], MBF16, name=f"XY{k}",
                                 tag=f"XY{k}") for k in range(NSLAB)]
            pus, pvs = [], []
            for k in range(NSLAB):
                pus.append(pstat.tile([NPART, SLAB], MBF16, name=f"PU{k}", tag=f"PU{k}"))
                pvs.append(pstat.tile([NPART, SLAB], MBF16, name=f"PV{k}", tag=f"PV{k}"))

            for k in range(NSLAB):
                sl = slice(k * SLAB, (k + 1) * SLAB)
                mb = m_bufs[k]
                nc.sync.dma_start(mb[:, S_ONE * SLAB:(S_ONE + 1) * SLAB], st8_d.ap()[:, sl])
                nc.sync.dma_start(pus[k][:], st16_d.ap()[0, :, sl])
                nc.sync.dma_start(pvs[k][:], st16_d.ap()[1, :, sl])

            ps_tiles = [pps.tile([SCOL, MCOL], F32, name=f"PS{s}", tag=f"PS{s}")
                        for s in range(SPC)]
            delta_t = pstat.tile([NPART, 1], F32, name="delta", tag="delta")
            nc.vector.memset(delta_t[:], DELTA)
            if skip:
                for k in range(NSLAB):
                    if "mask" in skip or "dma" in skip:
                        nc.vector.memset(oh_bufs[k][:], 0.0)
                    if "dma" in skip:
                        nc.vector.memset(xy_bufs[k][:], 1.0)
                        nc.vector.memset(m_bufs[k][:, S_Q * SLAB:(S_Q + 8) * SLAB], 0.5)
                    if "chan" in skip:
                        nc.vector.memset(m_bufs[k][:, S_DXH2 * SLAB:NCH * SLAB], 0.5)
                if "mm" in skip:
                    for s in range(SPC):
                        nc.vector.memset(ps_tiles[s][:], 0.0)

            loop_cm = (tc.For_i(0, reps // unroll, 1) if reps > 1
                       else contextlib.nullcontext())
            with loop_cm:
             for _u in range(unroll):
              for s in range(SPC):
                for k in range(NSLAB):
                    sl = slice(k * SLAB, (k + 1) * SLAB)
                    mb = m_bufs[k]
                    # --- loads (1 xy DMA, 1 feat DMA, 1 one-hot DMA) ---
                    # one-hot masks come precomputed from the host in
                    # chunk-major layout (col = chunk*C1 + (c-1)): each GRP
                    # group is a contiguous 120-col stationary run for the PE
                    xyt = xy_bufs[k]
                    oh = oh_bufs[k]
                    if "dma" not in skip:
                        # xy on its own queue so the channel chain starts
                        # without queuing behind the bulk feat/ohm transfers
                        nc.scalar.dma_start(
                            xyt[:],
                            xy_d.ap()[s].rearrange("a p c -> p a c")[:, :, sl])
                        nc.sync.dma_start(
                            mb[:, S_Q * SLAB:(S_Q + 8) * SLAB],
                            feat_d.ap()[s].rearrange("a p c -> p a c")[:, :, sl],
                        )
                        if "mask" not in skip:
                            nc.gpsimd.dma_start(
                                oh[:],
                                ohm_d.ap()[s, :, k * C1 * SLAB:(k + 1) * C1 * SLAB])
                    x0 = xyt[:, 0:SLAB]
                    x1 = xyt[:, SLAB:2 * SLAB]

                    # --- per-pixel direction weights ---
                    if "chan" in skip:
                        if "mm" not in skip:
                            mv_co = mb[:].rearrange("p (c s) -> p c s", c=NCH)
                            for t in range(NG):
                                nc.tensor.matmul(
                                    ps_tiles[s][:, :],
                                    oh[:, t * GRP * C1:(t + 1) * GRP * C1],
                                    mv_co[:, :, t * GRP:(t + 1) * GRP],
                                    start=(k == 0 and t == 0),
                                    stop=(k == NSLAB - 1 and t == NG - 1),
                                    skip_group_check=True,
                                )
                        continue
                    dxh2 = mb[:, S_DXH2 * SLAB:(S_DXH2 + 1) * SLAB]
                    mm_ = mb[:, S_M * SLAB:(S_M + 1) * SLAB]
                    u_ = mb[:, S_U * SLAB:(S_U + 1) * SLAB]
                    v_ = mb[:, S_V * SLAB:(S_V + 1) * SLAB]
                    # n2 = x0^2+x1^2; rr = rsqrt(n2+delta); dxh = x0*rr,
                    # dyh = x1*rr; dxh2 = dxh^2; m = dxh*dyh;
                    # w = dyh*pu - dxh*pv; u = dyh*w; v' = dxh*w (= -v, host
                    # negates ry). All tensor_tensor on DVE (gpsimd is ~30x
                    # slower per op on TRN2 - keep it off the hot path).
                    sq = ptmp.tile([NPART, 2 * SLAB], MBF16, name=f"sq_{s}_{k}", tag="sq")
                    nc.scalar.square(sq[:], xyt[:])
                    sx = sq[:, 0:SLAB]
                    sy = sq[:, SLAB:2 * SLAB]
                    n2 = ptmp.tile([NPART, SLAB], MBF16, name=f"n2_{s}_{k}", tag="n2")
                    nc.vector.tensor_tensor(n2[:], sx, sy, op=AOT.add)
                    rr = ptmp.tile([NPART, SLAB], MBF16, name=f"rr_{s}_{k}", tag="rr")
                    nc.scalar.activation(rr[:], n2[:], ACTF.Abs_reciprocal_sqrt, bias=delta_t[:])
                    dxh = ptmp.tile([NPART, SLAB], MBF16, name=f"dxh_{s}_{k}", tag="dxh")
                    dyh = ptmp.tile([NPART, SLAB], MBF16, name=f"dyh_{s}_{k}", tag="dyh")
                    nc.vector.tensor_tensor(dxh[:], x0, rr[:], op=AOT.mult)
                    nc.vector.tensor_tensor(dyh[:], x1, rr[:], op=AOT.mult)
                    nc.scalar.square(dxh2, dxh[:])
                    nc.vector.tensor_tensor(mm_, dxh[:], dyh[:], op=AOT.mult)
                    # Hough rhs channels via w = dyh*pu - dxh*pv
                    wa = ptmp.tile([NPART, SLAB], MBF16, name=f"wa_{s}_{k}", tag="wa")
                    wb = ptmp.tile([NPART, SLAB], MBF16, name=f"wb_{s}_{k}", tag="wb")
                    wt = ptmp.tile([NPART, SLAB], MBF16, name=f"wt_{s}_{k}", tag="wt")
                    nc.vector.tensor_tensor(wa[:], dyh[:], pus[k][:], op=AOT.mult)
                    nc.vector.tensor_tensor(wb[:], dxh[:], pvs[k][:], op=AOT.mult)
                    nc.vector.tensor_tensor(wt[:], wa[:], wb[:], op=AOT.subtract)
                    nc.vector.tensor_tensor(u_, dyh[:], wt[:], op=AOT.mult)
                    nc.vector.tensor_tensor(v_, dxh[:], wt[:], op=AOT.mult)

                    # --- PE segmented-sum stream: one matmul per GRP chunks;
                    # moving is channel-outer [128, NCH, GRP] (contig inner) ---
                    mv_co = mb[:].rearrange("p (c s) -> p c s", c=NCH)
                    if "mm" not in skip:
                        for t in range(NG):
                            nc.tensor.matmul(
                                ps_tiles[s][:, :],
                                oh[:, t * GRP * C1:(t + 1) * GRP * C1],
                                mv_co[:, :, t * GRP:(t + 1) * GRP],
                                start=(k == 0 and t == 0),
                                stop=(k == NSLAB - 1 and t == NG - 1),
                                skip_group_check=True,
                            )

            outs = ptmp.tile([SCOL, SPC * MCOL], F32)
            for s in range(SPC):
                nc.vector.tensor_copy(outs[:, s * MCOL:(s + 1) * MCOL], ps_tiles[s][:])
                nc.sync.dma_start(sums_d.ap()[s], outs[:, s * MCOL:(s + 1) * MCOL])

    nc.compile()
    _NC_CACHE[key] = nc
    return nc


def _host_prep(inputs):
    """Build per-core input maps (bf16 planes in [128, 2400] partition-major layout)."""
    cat = np.asarray(inputs["cat_mask"])
    quat = np.asarray(inputs["quaternion"], dtype=np.float32)
    scales = np.asarray(inputs["scales"], dtype=np.float32)
    xy = np.asarray(inputs["xy"], dtype=np.float32)
    z = np.asarray(inputs["z"], dtype=np.float32)

    st16, st8 = _build_static()

    feat = np.concatenate(
        [quat.reshape(B, 4, H * W), scales.reshape(B, 3, H * W),
         z.reshape(B, 1, H * W)], axis=1,
    ).reshape(B, 8, NPART, COLS).astype(FP8E4)
    xy16 = xy.reshape(B, 2, NPART, COLS).astype(BF16)
    # chunk-major one-hot masks [B, 128, 2400*6]: col = chunk*C1 + (c-1)
    cat_p = cat.reshape(B, NPART, COLS)
    ohm = (cat_p[..., None] == np.arange(1, CLASSES).reshape(1, 1, 1, C1)
           ).astype(FP8E4).reshape(B, NPART, COLS * C1)

    in_maps = []
    for i in range(NCORES):
        sl = slice(i * SPC, (i + 1) * SPC)
        in_maps.append({
            "feat": np.ascontiguousarray(feat[sl]),
            "ohm": np.ascontiguousarray(ohm[sl]),
            "xy": np.ascontiguousarray(xy16[sl]),
            "st16": st16,
            "st8": st8,
        })
    return in_maps


def _host_finish(sums_all, intrinsics):
    """sums_all: [B, C1, NCH] float64. Returns [C1, B, 26] float32."""
    S = sums_all
    cnt = S[..., S_ONE]
    denom = np.maximum(cnt, 1.0)
    q_agg = S[..., S_Q:S_Q + 4] / denom[..., None]
    s_agg = S[..., S_S:S_S + 3] / denom[..., None]
    z_agg = S[..., S_Z] / denom

    # dxh2+dyh2 == 1 per pixel, so Ayy = sum(mask*dxh2) directly
    Axx = cnt - S[..., S_DXH2]
    Ayy = S[..., S_DXH2]
    Axy = -S[..., S_M]
    rx = S[..., S_U] * 4.0        # pu/pv were pre-scaled by 1/4
    ry = -S[..., S_V] * 4.0       # device stores v' = -v/4

    A = np.empty(S.shape[:2] + (2, 2))
    A[..., 0, 0] = Axx + EPS
    A[..., 0, 1] = Axy
    A[..., 1, 0] = Axy
    A[..., 1, 1] = Ayy + EPS
    rhs = np.stack([rx, ry], axis=-1)
    center = np.linalg.solve(A, rhs[..., None])[..., 0]  # [B, C1, 2]

    qn = q_agg / (np.linalg.norm(q_agg, axis=-1, keepdims=True) + 1e-8)
    w, x, y, zz = qn[..., 0], qn[..., 1], qn[..., 2], qn[..., 3]
    R = np.stack([
        1 - 2 * (y * y + zz * zz), 2 * (x * y - w * zz), 2 * (x * zz + w * y),
        2 * (x * y + w * zz), 1 - 2 * (x * x + zz * zz), 2 * (y * zz - w * x),
        2 * (x * zz - w * y), 2 * (y * zz + w * x), 1 - 2 * (x * x + y * y),
    ], axis=-1).reshape(S.shape[:2] + (3, 3))

    zval = np.exp(z_agg)
    Kinv = np.linalg.inv(np.asarray(intrinsics, dtype=np.float64))
    homog = np.concatenate([center, np.ones(S.shape[:2] + (1,))], axis=-1)
    t = zval[..., None] * np.einsum("ij,bcj->bci", Kinv, homog)

    RT = np.zeros(S.shape[:2] + (4, 4))
    RT[..., :3, :3] = R
    RT[..., :3, 3] = t
    RT[..., 3, 3] = 1.0

    out = np.concatenate(
        [q_agg, s_agg, z_agg[..., None], center, RT.reshape(S.shape[:2] + (16,))],
        axis=-1,
    )  # [B, C1, 26]
    return np.transpose(out, (1, 0, 2)).astype(np.float32)


def kernel(**inputs):
    from concourse.bass_utils import run_bass_kernel_spmd

    nc = _build_nc()
    in_maps = _host_prep(inputs)
    res = run_bass_kernel_spmd(nc, in_maps, core_ids=list(range(NCORES)))
    sums_all = np.empty((B, C1, NCH), dtype=np.float64)
    for i in range(NCORES):
        r = res.results[i]["sums"].astype(np.float64)  # [SPC, GRP*C1, NCH*GRP]
        r = r.reshape(SPC, GRP, C1, NCH, GRP)
        diag = np.einsum("sgckg->sck", r)
        for j in range(SPC):
            sums_all[i * SPC + j] = diag[j]
    return _host_finish(sums_all, inputs["intrinsics"])


# revision 44
# speedup vs baseline: 1.1458x; 1.1458x over previous
"""Trainium2 Bass kernel for nn_AggregationLayer (per-class masked reductions + Hough voting).

Strategy (8 NeuronCores, data-parallel over batch: 2 samples/core):
  The device computes, per (class c in 1..6, sample b), 13 masked sums
      S_c[x] = sum_p [cat_p == c] * x_p
  over the 307200 pixels of each sample, for channels x in
      {1, q0..q3, s0..s2, z, dxh2, m, u, v}
  where dxh2 = dx^2, m = dx*dy (direction-normalized via rsqrt(|xy|^2+delta))
  are the Hough direction-matrix terms (dy^2 = 1 - dx^2 is folded into the
  host finish) and
      u = dy^2*pu - m*pv,  v = dx^2*pv - m*pu
  fold the position-weighted Hough rhs into single channels:
  rx = sum mask*u, ry = sum mask*v exactly.

  The segmented (per-class) reduction runs on the TensorEngine with G=20
  pixel-chunks fused per matmul: stationary = 20 chunks' one-hot columns
  [128, 120] (chunk-major layout -> contiguous run), moving = the 13 channel
  planes' 20-column runs as a 2D channel-outer AP [128, 13, 20] (innermost
  contiguous), accumulated into a [120, 260] PSUM tile whose (g,g) diagonal
  blocks are the wanted per-class sums (off-diagonal cross products are
  ignored). 240 matmuls/core at ~95 ns each; LDWEIGHTS overlaps streaming.

  One-hot masks are built in chunk-major layout with a SINGLE is_equal
  tensor_tensor per slab: cat broadcast along an inner 6-wide dim vs a
  periodic [1..6] class pattern - all accesses contiguous.

  The host does only the tiny [6, B] finalization: 2x2 solve for the Hough
  center, quaternion -> rotation matrix, intrinsics backprojection, packing
  into the [6, 16, 26] output.
"""

import numpy as np
import ml_dtypes

B, H, W = 16, 480, 640
CLASSES = 7
C1 = CLASSES - 1
NCORES = 8
SPC = B // NCORES          # samples per core
NPART = 128
COLS = (H * W) // NPART    # 2400
SLAB = 600
NSLAB = COLS // SLAB       # 3
NCH = 13                   # moving channels
GRP = 20                   # pixel chunks fused per matmul
DELTA = 1e-12              # guard for 1/(n2 + DELTA)
EPS = 1e-6                 # matches reference

BF16 = ml_dtypes.bfloat16
FP8E4 = ml_dtypes.float8_e4m3fn

# moving-channel slot map
S_ONE, S_Q, S_S, S_Z = 0, 1, 5, 8
S_DXH2, S_M = 9, 10
S_U, S_V = 11, 12

_NC_CACHE = {}
_STATIC_CACHE = {}


def _build_static():
    if "st" in _STATIC_CACHE:
        return _STATIC_CACHE["st"]
    p = np.arange(H * W, dtype=np.int64)
    # pu/pv pre-scaled by 1/4 so |u| = |dy^2*pu - m*pv| fits fp8e4's range;
    # the host finish multiplies rx/ry back by 4
    pu = (p % W).astype(np.float64) * 0.25
    pv = (p // W).astype(np.float64) * 0.25
    ones = np.ones_like(pu)

    def plane16(a):
        return a.reshape(NPART, COLS).astype(BF16)

    st16 = np.stack([plane16(pu), plane16(pv)])  # [2,128,2400] bf16
    st8 = ones.reshape(NPART, COLS).astype(FP8E4)
    _STATIC_CACHE["st"] = (st16, st8)
    return st16, st8


def _build_nc(reps=1, skip=(), unroll=1):
    """Build + compile the SPMD Bass program. reps > 1 wraps the whole
    pipeline in a hardware For loop (used only for benchmarking; the loop
    body holds `unroll` reps and the loop runs reps//unroll times).
    skip: subset of {"mask","chan","mm","dma"} disabling stages (timing only)."""
    skip = frozenset(skip)
    key = (reps, SLAB, GRP, NCH, skip, unroll)
    if key in _NC_CACHE:
        return _NC_CACHE[key]
    import contextlib
    import concourse.bacc as bacc
    import concourse.mybir as mybir
    import concourse.tile as tile

    F32, MBF16 = mybir.dt.float32, mybir.dt.bfloat16
    FP8 = mybir.dt.float8e4
    AOT = mybir.AluOpType
    ACTF = mybir.ActivationFunctionType

    NG = SLAB // GRP           # matmul groups per slab
    SCOL = GRP * C1            # stationary columns (120)
    MCOL = GRP * NCH           # moving columns (260)

    nc = bacc.Bacc("TRN2", target_bir_lowering=False, debug=False)
    feat_d = nc.dram_tensor("feat", [SPC, 8, NPART, COLS], FP8, kind="ExternalInput")
    ohm_d = nc.dram_tensor("ohm", [SPC, NPART, C1 * H * W // NPART], FP8,
                           kind="ExternalInput")
    xy_d = nc.dram_tensor("xy", [SPC, 2, NPART, COLS], MBF16, kind="ExternalInput")
    st8_d = nc.dram_tensor("st8", [NPART, COLS], FP8, kind="ExternalInput")
    st16_d = nc.dram_tensor("st16", [2, NPART, COLS], MBF16, kind="ExternalInput")
    sums_d = nc.dram_tensor("sums", [SPC, SCOL, MCOL], F32, kind="ExternalOutput")

    with tile.TileContext(nc) as tc:
        with (
            tc.tile_pool(name="mov", bufs=1) as pmov,
            tc.tile_pool(name="stat", bufs=1) as pstat,
            tc.tile_pool(name="tmp", bufs=3) as ptmp,
            tc.tile_pool(name="psum", bufs=1, space="PSUM") as pps,
        ):
            # persistent buffers, one per slab phase (3-deep rotation);
            # the static ones channel is written once per physical buffer and
            # survives the per-sample rewrites of slots 1-12
            m_bufs = [pmov.tile([NPART, NCH * SLAB], FP8, name=f"Mbuf{k}",
                                tag=f"Mbuf{k}") for k in range(NSLAB)]
            oh_bufs = [pmov.tile([NPART, C1 * SLAB], FP8, name=f"OH{k}",
                                 tag=f"OH{k}") for k in range(NSLAB)]
            xy_bufs = [pmov.tile([NPART, 2 * SLAB], MBF16, name=f"XY{k}",
                                 tag=f"XY{k}") for k in range(NSLAB)]
            pus, pvs = [], []
            for k in range(NSLAB):
                pus.append(pstat.tile([NPART, SLAB], MBF16, name=f"PU{k}", tag=f"PU{k}"))
                pvs.append(pstat.tile([NPART, SLAB], MBF16, name=f"PV{k}", tag=f"PV{k}"))

            for k in range(NSLAB):
                sl = slice(k * SLAB, (k + 1) * SLAB)
                mb = m_bufs[k]
                nc.sync.dma_start(mb[:, S_ONE * SLAB:(S_ONE + 1) * SLAB], st8_d.ap()[:, sl])
                nc.sync.dma_start(pus[k][:], st16_d.ap()[0, :, sl])
                nc.sync.dma_start(pvs[k][:], st16_d.ap()[1, :, sl])

            ps_tiles = [pps.tile([SCOL, MCOL], F32, name=f"PS{s}", tag=f"PS{s}")
                        for s in range(SPC)]
            delta_t = pstat.tile([NPART, 1], F32, name="delta", tag="delta")
            nc.vector.memset(delta_t[:], DELTA)
            if skip:
                for k in range(NSLAB):
                    if "mask" in skip or "dma" in skip:
                        nc.vector.memset(oh_bufs[k][:], 0.0)
                    if "dma" in skip:
                        nc.vector.memset(xy_bufs[k][:], 1.0)
                        nc.vector.memset(m_bufs[k][:, S_Q * SLAB:(S_Q + 8) * SLAB], 0.5)
                    if "chan" in skip:
                        nc.vector.memset(m_bufs[k][:, S_DXH2 * SLAB:NCH * SLAB], 0.5)
                if "mm" in skip:
                    for s in range(SPC):
                        nc.vector.memset(ps_tiles[s][:], 0.0)

            loop_cm = (tc.For_i(0, reps // unroll, 1) if reps > 1
                       else contextlib.nullcontext())
            with loop_cm:
             for _u in range(unroll):
              for s in range(SPC):
                for k in range(NSLAB):
                    sl = slice(k * SLAB, (k + 1) * SLAB)
                    mb = m_bufs[k]
                    # --- loads (1 xy DMA, 1 feat DMA, 1 one-hot DMA) ---
                    # one-hot masks come precomputed from the host in
                    # chunk-major layout (col = chunk*C1 + (c-1)): each GRP
                    # group is a contiguous 120-col stationary run for the PE
                    xyt = xy_bufs[k]
                    oh = oh_bufs[k]
                    if "dma" not in skip:
                        # xy on its own queue so the channel chain starts
                        # without queuing behind the bulk feat/ohm transfers
                        nc.scalar.dma_start(
                            xyt[:],
                            xy_d.ap()[s].rearrange("a p c -> p a c")[:, :, sl])
                        nc.sync.dma_start(
                            mb[:, S_Q * SLAB:(S_Q + 8) * SLAB],
                            feat_d.ap()[s].rearrange("a p c -> p a c")[:, :, sl],
                        )
                        if "mask" not in skip:
                            nc.gpsimd.dma_start(
                                oh[:],
                                ohm_d.ap()[s, :, k * C1 * SLAB:(k + 1) * C1 * SLAB])
                    x0 = xyt[:, 0:SLAB]
                    x1 = xyt[:, SLAB:2 * SLAB]

                    # --- per-pixel direction weights ---
                    if "chan" in skip:
                        if "mm" not in skip:
                            mv_co = mb[:].rearrange("p (c s) -> p c s", c=NCH)
                            for t in range(NG):
                                nc.tensor.matmul(
                                    ps_tiles[s][:, :],
                                    oh[:, t * GRP * C1:(t + 1) * GRP * C1],
                                    mv_co[:, :, t * GRP:(t + 1) * GRP],
                                    start=(k == 0 and t == 0),
                                    stop=(k == NSLAB - 1 and t == NG - 1),
                                    skip_group_check=True,
                                )
                        continue
                    dxh2 = mb[:, S_DXH2 * SLAB:(S_DXH2 + 1) * SLAB]
                    mm_ = mb[:, S_M * SLAB:(S_M + 1) * SLAB]
                    u_ = mb[:, S_U * SLAB:(S_U + 1) * SLAB]
                    v_ = mb[:, S_V * SLAB:(S_V + 1) * SLAB]
                    # n2 = x0^2+x1^2; rr = rsqrt(n2+delta); dxh = x0*rr,
                    # dyh = x1*rr; dxh2 = dxh^2; m = dxh*dyh;
                    # w = dyh*pu - dxh*pv; u = dyh*w; v' = dxh*w (= -v, host
                    # negates ry). All tensor_tensor on DVE (gpsimd is ~30x
                    # slower per op on TRN2 - keep it off the hot path).
                    sq = ptmp.tile([NPART, 2 * SLAB], MBF16, name=f"sq_{s}_{k}", tag="sq")
                    nc.scalar.square(sq[:], xyt[:])
                    sx = sq[:, 0:SLAB]
                    sy = sq[:, SLAB:2 * SLAB]
                    n2 = ptmp.tile([NPART, SLAB], MBF16, name=f"n2_{s}_{k}", tag="n2")
                    nc.vector.tensor_tensor(n2[:], sx, sy, op=AOT.add)
                    rr = ptmp.tile([NPART, SLAB], MBF16, name=f"rr_{s}_{k}", tag="rr")
                    nc.scalar.activation(rr[:], n2[:], ACTF.Abs_reciprocal_sqrt, bias=delta_t[:])
                    dxh = ptmp.tile([NPART, SLAB], MBF16, name=f"dxh_{s}_{k}", tag="dxh")
                    dyh = ptmp.tile([NPART, SLAB], MBF16, name=f"dyh_{s}_{k}", tag="dyh")
                    nc.vector.tensor_tensor(dxh[:], x0, rr[:], op=AOT.mult)
                    nc.vector.tensor_tensor(dyh[:], x1, rr[:], op=AOT.mult)
                    nc.scalar.square(dxh2, dxh[:])
                    nc.vector.tensor_tensor(mm_, dxh[:], dyh[:], op=AOT.mult)
                    # Hough rhs channels via w = dyh*pu - dxh*pv
                    wa = ptmp.tile([NPART, SLAB], MBF16, name=f"wa_{s}_{k}", tag="wa")
                    wb = ptmp.tile([NPART, SLAB], MBF16, name=f"wb_{s}_{k}", tag="wb")
                    wt = ptmp.tile([NPART, SLAB], MBF16, name=f"wt_{s}_{k}", tag="wt")
                    nc.vector.tensor_tensor(wa[:], dyh[:], pus[k][:], op=AOT.mult)
                    nc.vector.tensor_tensor(wb[:], dxh[:], pvs[k][:], op=AOT.mult)
                    nc.vector.tensor_tensor(wt[:], wa[:], wb[:], op=AOT.subtract)
                    nc.vector.tensor_tensor(u_, dyh[:], wt[:], op=AOT.mult)
                    nc.vector.tensor_tensor(v_, dxh[:], wt[:], op=AOT.mult)

                    # --- PE segmented-sum stream: one matmul per GRP chunks;
                    # moving is channel-outer [128, NCH, GRP] (contig inner) ---
                    mv_co = mb[:].rearrange("p (c s) -> p c s", c=NCH)
                    if "mm" not in skip:
                        for t in range(NG):
                            nc.tensor.matmul(
                                ps_tiles[s][:, :],
                                oh[:, t * GRP * C1:(t + 1) * GRP * C1],
                                mv_co[:, :, t * GRP:(t + 1) * GRP],
                                start=(k == 0 and t == 0),
                                stop=(k == NSLAB - 1 and t == NG - 1),
                                skip_group_check=True,
                            )

            outs = ptmp.tile([SCOL, SPC * MCOL], F32)
            for s in range(SPC):
                nc.vector.tensor_copy(outs[:, s * MCOL:(s + 1) * MCOL], ps_tiles[s][:])
                nc.sync.dma_start(sums_d.ap()[s], outs[:, s * MCOL:(s + 1) * MCOL])

    nc.compile()
    _NC_CACHE[key] = nc
    return nc


def _host_prep(inputs):
    """Build per-core input maps (bf16 planes in [128, 2400] partition-major layout)."""
    cat = np.asarray(inputs["cat_mask"])
    quat = np.asarray(inputs["quaternion"], dtype=np.float32)
    scales = np.asarray(inputs["scales"], dtype=np.float32)
    xy = np.asarray(inputs["xy"], dtype=np.float32)
    z = np.asarray(inputs["z"], dtype=np.float32)

    st16, st8 = _build_static()

    feat = np.concatenate(
        [quat.reshape(B, 4, H * W), scales.reshape(B, 3, H * W),
         z.reshape(B, 1, H * W)], axis=1,
    ).reshape(B, 8, NPART, COLS).astype(FP8E4)
    xy16 = xy.reshape(B, 2, NPART, COLS).astype(BF16)
    # chunk-major one-hot masks [B, 128, 2400*6]: col = chunk*C1 + (c-1)
    cat_p = cat.reshape(B, NPART, COLS)
    ohm = (cat_p[..., None] == np.arange(1, CLASSES).reshape(1, 1, 1, C1)
           ).astype(FP8E4).reshape(B, NPART, COLS * C1)

    in_maps = []
    for i in range(NCORES):
        sl = slice(i * SPC, (i + 1) * SPC)
        in_maps.append({
            "feat": np.ascontiguousarray(feat[sl]),
            "ohm": np.ascontiguousarray(ohm[sl]),
            "xy": np.ascontiguousarray(xy16[sl]),
            "st16": st16,
            "st8": st8,
        })
    return in_maps


def _host_finish(sums_all, intrinsics):
    """sums_all: [B, C1, NCH] float64. Returns [C1, B, 26] float32."""
    S = sums_all
    cnt = S[..., S_ONE]
    denom = np.maximum(cnt, 1.0)
    q_agg = S[..., S_Q:S_Q + 4] / denom[..., None]
    s_agg = S[..., S_S:S_S + 3] / denom[..., None]
    z_agg = S[..., S_Z] / denom

    # dxh2+dyh2 == 1 per pixel, so Ayy = sum(mask*dxh2) directly
    Axx = cnt - S[..., S_DXH2]
    Ayy = S[..., S_DXH2]
    Axy = -S[..., S_M]
    rx = S[..., S_U] * 4.0        # pu/pv were pre-scaled by 1/4
    ry = -S[..., S_V] * 4.0       # device stores v' = -v/4

    A = np.empty(S.shape[:2] + (2, 2))
    A[..., 0, 0] = Axx + EPS
    A[..., 0, 1] = Axy
    A[..., 1, 0] = Axy
    A[..., 1, 1] = Ayy + EPS
    rhs = np.stack([rx, ry], axis=-1)
    center = np.linalg.solve(A, rhs[..., None])[..., 0]  # [B, C1, 2]

    qn = q_agg / (np.linalg.norm(q_agg, axis=-1, keepdims=True) + 1e-8)
    w, x, y, zz = qn[..., 0], qn[..., 1], qn[..., 2], qn[..., 3]
    R = np.stack([
        1 - 2 * (y * y + zz * zz), 2 * (x * y - w * zz), 2 * (x * zz + w * y),
        2 * (x * y + w * zz), 1 - 2 * (x * x + zz * zz), 2 * (y * zz - w * x),
        2 * (x * zz - w * y), 2 * (y * zz + w * x), 1 - 2 * (x * x + y * y),
    ], axis=-1).reshape(S.shape[:2] + (3, 3))

    zval = np.exp(z_agg)
    Kinv = np.linalg.inv(np.asarray(intrinsics, dtype=np.float64))
    homog = np.concatenate([center, np.ones(S.shape[:2] + (1,))], axis=-1)
    t = zval[..., None] * np.einsum("ij,bcj->bci", Kinv, homog)

    RT = np.zeros(S.shape[:2] + (4, 4))
    RT[..., :3, :3] = R
    RT[..., :3, 3] = t
    RT[..., 3, 3] = 1.0

    out = np.concatenate(
        [q_agg, s_agg, z_agg[..., None], center, RT.reshape(S.shape[:2] + (16,))],
        axis=-1,
    )  # [B, C1, 26]
    return np.transpose(out, (1, 0, 2)).astype(np.float32)


def kernel(**inputs):
    from concourse.bass_utils import run_bass_kernel_spmd

    nc = _build_nc()
    in_maps = _host_prep(inputs)
    res = run_bass_kernel_spmd(nc, in_maps, core_ids=list(range(NCORES)))
    sums_all = np.empty((B, C1, NCH), dtype=np.float64)
    for i in range(NCORES):
        r = res.results[i]["sums"].astype(np.float64)  # [SPC, GRP*C1, NCH*GRP]
        r = r.reshape(SPC, GRP, C1, NCH, GRP)
        diag = np.einsum("sgckg->sck", r)
        for j in range(SPC):
            sums_all[i * SPC + j] = diag[j]
    return _host_finish(sums_all, inputs["intrinsics"])
